# revision 45
# baseline (speedup 1.0000x reference)
"""MedianPool2d (3x3, stride 1, zero-pad 1) Trainium2 Bass kernel.

Full input x: (8, 64, 256, 256) fp32.  Sharding: pure data parallel over
batch -> core i processes x[i] (64, 256, 256).

Per-core layout: 128 SBUF partitions = (h, c) with p = h*64 + c, where
h in {0,1} picks the top/bottom 128-row half of the image and c the
channel.  Each partition processes a strip of HH=128 rows x 256 cols,
with a 1-row halo on each side (zero at the image border, neighbor rows
at the half boundary - both come in via DMA / memset).

Median of 9 = med3(max3(column mins), med3(column medians),
                   min3(column maxes))  -- 15 min/max passes/pixel with
vertical row-pair sharing and horizontal even/odd pair sharing.  All
elementwise work runs on the DVE; this toolchain rejects TensorTensor /
tensor_max on GPSIMD and CCE min/max accum on DMA, so the DVE is the
only min/max engine and the kernel is DVE-throughput-bound.

The fast path (_build_fp16_split) computes in fp16 for the DVE's 2x_1p
perf mode (2 elem/cycle; requires 2-byte dtype + stride-1 last AP dim on
every operand), storing each padded row as contiguous even/odd column
planes so the horizontal pair sharing stays stride-1.  The Activation
engine does the fp32<->fp16 casts and the (de)interleave.  See
_build_fp16_split's docstring for details.  546.7us (fp32 baseline)
-> ~284us; DVE busy is ~95% of the timeline.
"""

import numpy as np

B, C, H, W = 8, 64, 256, 256
NCORES = 8
HH = H // 2          # rows per half-strip
WP = W + 2           # padded row width

_CACHE = {}


def _build(R=8, gp_rows=0):
    """Build the Bass module for one core: x (64,256,256) f32 -> out same.

    gp_rows: number of output rows (of each chunk's R) computed on GPSIMD
    instead of the DVE.
    """
    import concourse.bacc as bacc
    import concourse.mybir as mybir
    from concourse.tile import TileContext

    MIN = mybir.AluOpType.min
    MAX = mybir.AluOpType.max
    f32 = mybir.dt.float32

    assert HH % R == 0
    assert 0 <= gp_rows < R
    K = HH // R                     # chunks per strip

    nc = bacc.Bacc("TRN2", name="median_pool2d")
    x = nc.dram_tensor("x", [C, H, W], f32, kind="ExternalInput")
    out = nc.dram_tensor("out", [C, H, W], f32, kind="ExternalOutput")

    xg = x.ap()                     # global view [c, 256, 256]
    og = out.ap()

    def tt(out_ap, in0, in1, op):
        """Elementwise tensor_tensor, row-split DVE/GPSIMD.

        All APs are [128, rows, width]; the row dim is axis 1.
        """
        rows = out_ap.shape[1]
        split = rows - gp_rows if rows > gp_rows else rows
        nc.vector.tensor_tensor(
            out=out_ap[:, 0:split], in0=in0[:, 0:split], in1=in1[:, 0:split],
            op=op,
        )
        if split < rows:
            nc.gpsimd.tensor_tensor(
                out=out_ap[:, split:rows], in0=in0[:, split:rows],
                in1=in1[:, split:rows], op=op,
            )

    with TileContext(nc) as tc:
        with (
            tc.tile_pool(name="io_in", bufs=3) as in_pool,
            tc.tile_pool(name="io_out", bufs=3) as out_pool,
            tc.tile_pool(name="vert", bufs=1) as v_pool,
            tc.tile_pool(name="merge", bufs=1) as m_pool,
        ):
            for k in range(K):
                r0 = k * R                      # first output row (half-local)
                # ---- load input chunk: rows r0-1 .. r0+R (R+2 rows) ----
                it = in_pool.tile([128, (R + 2) * WP], f32, name="it", tag="it")
                it3 = it.rearrange("p (r w) -> p r w", w=WP)
                # zero pad columns 0 and 257 for all rows
                nc.vector.memset(it3[:, :, 0:WP:WP - 1], 0.0)
                # top half: global rows r0-1 .. r0+R+1 (clip at k==0)
                if k == 0:
                    nc.vector.memset(it3[0:64, 0:1, 1:W + 1], 0.0)
                    nc.sync.dma_start(
                        out=it3[0:64, 1:R + 2, 1:W + 1],
                        in_=xg[:, 0:R + 1, :],
                    )
                else:
                    nc.sync.dma_start(
                        out=it3[0:64, :, 1:W + 1],
                        in_=xg[:, r0 - 1:r0 + R + 1, :],
                    )
                # bottom half: global rows HH+r0-1 .. HH+r0+R+1 (clip at last)
                if k == K - 1:
                    nc.vector.memset(it3[64:128, R + 1:R + 2, 1:W + 1], 0.0)
                    nc.sync.dma_start(
                        out=it3[64:128, 0:R + 1, 1:W + 1],
                        in_=xg[:, HH + r0 - 1:H, :],
                    )
                else:
                    nc.sync.dma_start(
                        out=it3[64:128, :, 1:W + 1],
                        in_=xg[:, HH + r0 - 1:HH + r0 + R + 1, :],
                    )

                # ---- vertical sort3 over rows (full padded width) ----
                X0 = it3[:, 0:R, :]
                X1 = it3[:, 1:R + 1, :]
                X2 = it3[:, 2:R + 2, :]

                def vtile(name):
                    t = v_pool.tile([128, R * WP], f32, name=name, tag=name)
                    return t.rearrange("p (r w) -> p r w", w=WP)

                P3 = vtile("bP")
                Q3 = vtile("bQ")
                Lo3 = vtile("bLo")
                W3 = vtile("bW")
                Me3 = vtile("bMe")
                Hi3 = vtile("bHi")

                tt(P3, X0, X1, MIN)
                tt(Q3, X0, X1, MAX)
                tt(Lo3, P3, X2, MIN)
                tt(W3, Q3, X2, MIN)
                tt(Me3, P3, W3, MAX)
                tt(Hi3, Q3, X2, MAX)

                # ---- horizontal merge (width 256 of 258) ----
                lo = [Lo3[:, :, d:d + W] for d in range(3)]
                me = [Me3[:, :, d:d + W] for d in range(3)]
                hi = [Hi3[:, :, d:d + W] for d in range(3)]

                def mtile(name):
                    t = m_pool.tile([128, R * W], f32, name=name, tag=name)
                    return t.rearrange("p (r w) -> p r w", w=W)

                mA = mtile("mA")
                mC = mtile("mC")
                mB = mtile("mB")
                mT = mtile("mT")
                mU = mtile("mU")
                mV = mtile("mV")

                # A = max3(lo)
                tt(mT, lo[0], lo[1], MAX)
                tt(mA, mT, lo[2], MAX)
                # C = min3(hi)
                tt(mU, hi[0], hi[1], MIN)
                tt(mC, mU, hi[2], MIN)
                # B = med3(me) = max(min(a,b), min(max(a,b), c))
                tt(mT, me[0], me[1], MIN)
                tt(mU, me[0], me[1], MAX)
                tt(mV, mU, me[2], MIN)
                tt(mB, mT, mV, MAX)

                # out = med3(A, B, C)
                ot = out_pool.tile([128, R * W], f32, name="ot", tag="ot")
                ot3 = ot.rearrange("p (r w) -> p r w", w=W)
                tt(mT, mA, mB, MIN)
                tt(mU, mA, mB, MAX)
                tt(mV, mU, mC, MIN)
                tt(ot3, mT, mV, MAX)

                # ---- store ----
                nc.sync.dma_start(out=og[:, r0:r0 + R, :], in_=ot3[0:64])
                nc.sync.dma_start(
                    out=og[:, HH + r0:HH + r0 + R, :], in_=ot3[64:128]
                )

    nc.compile()
    return nc


def _build_shared(R=8, gp_frac=0.0, dtype="float32", in_bufs=None, out_bufs=None):
    """15-op/pixel variant: vertical pair sharing + horizontal even/odd
    pair sharing in the merge.  gp_frac: fraction of rows of every
    elementwise op executed on GPSIMD instead of the DVE (unsupported by
    the current toolchain - keep 0).  dtype: compute dtype on-chip;
    float16 doubles DVE throughput on step-1 ops at ~2e-4 max rel err."""
    import concourse.bacc as bacc
    import concourse.mybir as mybir
    from concourse.tile import TileContext

    MIN = mybir.AluOpType.min
    MAX = mybir.AluOpType.max
    f32 = mybir.dt.float32
    cdt = getattr(mybir.dt, dtype)
    cast = cdt != f32

    assert HH % R == 0 and R % 2 == 0
    K = HH // R
    Rh = R // 2

    nc = bacc.Bacc("TRN2", name="median_pool2d_s")
    x = nc.dram_tensor("x", [C, H, W], f32, kind="ExternalInput")
    out = nc.dram_tensor("out", [C, H, W], f32, kind="ExternalOutput")
    xg = x.ap()
    og = out.ap()
    dma_io = nc.gpsimd if cast else nc.sync

    def tt(out_ap, in0, in1, op):
        rows = out_ap.shape[1]
        gp = int(rows * gp_frac + 0.5)
        split = rows - gp
        if split > 0:
            nc.vector.tensor_tensor(
                out=out_ap[:, 0:split], in0=in0[:, 0:split],
                in1=in1[:, 0:split], op=op,
            )
        if split < rows:
            nc.gpsimd.tensor_tensor(
                out=out_ap[:, split:rows], in0=in0[:, split:rows],
                in1=in1[:, split:rows], op=op,
            )

    if in_bufs is None:
        in_bufs = 3 if R <= 8 else 2
    if out_bufs is None:
        out_bufs = 3 if R <= 8 else 1
    with TileContext(nc) as tc:
        with (
            tc.tile_pool(name="io_in", bufs=in_bufs) as in_pool,
            tc.tile_pool(name="io_out", bufs=out_bufs) as out_pool,
            tc.tile_pool(name="work", bufs=1) as w_pool,
        ):
            def wtile(name, rows, width, tag=None):
                t = w_pool.tile([128, rows * width], cdt, name=name,
                                tag=tag or name)
                return t.rearrange("p (r w) -> p r w", w=width)

            for k in range(K):
                r0 = k * R
                it = in_pool.tile([128, (R + 2) * WP], cdt, name="it", tag="it")
                it3 = it.rearrange("p (r w) -> p r w", w=WP)
                nc.vector.memset(it3[:, :, 0:WP:WP - 1], 0.0)
                if k == 0:
                    nc.vector.memset(it3[0:64, 0:1, 1:W + 1], 0.0)
                    dma_io.dma_start(out=it3[0:64, 1:R + 2, 1:W + 1],
                                      in_=xg[:, 0:R + 1, :])
                else:
                    dma_io.dma_start(out=it3[0:64, :, 1:W + 1],
                                      in_=xg[:, r0 - 1:r0 + R + 1, :])
                if k == K - 1:
                    nc.vector.memset(it3[64:128, R + 1:R + 2, 1:W + 1], 0.0)
                    dma_io.dma_start(out=it3[64:128, 0:R + 1, 1:W + 1],
                                      in_=xg[:, HH + r0 - 1:H, :])
                else:
                    dma_io.dma_start(out=it3[64:128, :, 1:W + 1],
                                      in_=xg[:, HH + r0 - 1:HH + r0 + R + 1, :])

                # ---- vertical: shared pair sort ----
                # pairs over in-tile row pairs (2i+1, 2i+2), i = 0..R/2-1
                Pm = wtile("Pm", Rh, WP)
                PM = wtile("PM", Rh, WP)
                tt(Pm, it3[:, 1:R + 1:2, :], it3[:, 2:R + 2:2, :], MIN)
                tt(PM, it3[:, 1:R + 1:2, :], it3[:, 2:R + 2:2, :], MAX)

                Lo3 = wtile("Lo", R, WP)
                Me3 = wtile("Me", R, WP)
                Hi3 = wtile("Hi", R, WP)
                tE = wtile("tE", Rh, WP)
                tO = wtile("tO", Rh, WP)
                a_e = it3[:, 0:R:2, :]          # third element, even out rows
                a_o = it3[:, 3:R + 2:2, :]      # rows 3,5,..,R+1 (count R/2)
                # even out rows y=0,2,..  (pair index i=y/2)
                tt(Lo3[:, 0:R:2], a_e, Pm, MIN)
                tt(Hi3[:, 0:R:2], a_e, PM, MAX)
                tt(tE, a_e, PM, MIN)
                tt(Me3[:, 0:R:2], Pm, tE, MAX)
                # odd out rows y=1,3,..   (pair index i=(y-1)/2)
                tt(Lo3[:, 1:R:2], a_o, Pm, MIN)
                tt(Hi3[:, 1:R:2], a_o, PM, MAX)
                tt(tO, a_o, PM, MIN)
                tt(Me3[:, 1:R:2], Pm, tO, MAX)

                # ---- merge: horizontal shared pairs ----
                NP = W // 2 + 1                 # 129 pairs over padded width
                # Pm/PM/tE/tO are dead after the vertical completions;
                # alias their slots (Rh*WP = 2064 >= R*NP = 2064 elems).
                PA = wtile("PA", R, NP, tag="Pm")
                PC = wtile("PC", R, NP, tag="PM")
                Um = wtile("Um", R, NP, tag="tE")
                Vm = wtile("Vm", R, NP, tag="tO")
                # PA/PC (in Pm/PM slots) are dead once mA/mC are built;
                # rotate tBe/tBo through the same slots.
                tBe = wtile("tBe", R, W // 2, tag="Pm")
                tBo = wtile("tBo", R, W // 2, tag="PM")
                mA = wtile("mA", R, W)
                mB = wtile("mB", R, W)
                mC = wtile("mC", R, W)

                ev = slice(0, WP, 2)            # padded even cols (129)
                od = slice(1, WP, 2)            # padded odd cols (129)
                tt(PA, Lo3[:, :, ev], Lo3[:, :, od], MAX)
                tt(mA[:, :, 0:W:2], PA[:, :, 0:NP - 1], Lo3[:, :, 2:WP:2], MAX)
                tt(mA[:, :, 1:W:2], PA[:, :, 1:NP], Lo3[:, :, 1:WP - 2:2], MAX)

                tt(PC, Hi3[:, :, ev], Hi3[:, :, od], MIN)
                tt(mC[:, :, 0:W:2], PC[:, :, 0:NP - 1], Hi3[:, :, 2:WP:2], MIN)
                tt(mC[:, :, 1:W:2], PC[:, :, 1:NP], Hi3[:, :, 1:WP - 2:2], MIN)

                tt(Um, Me3[:, :, ev], Me3[:, :, od], MIN)
                tt(Vm, Me3[:, :, ev], Me3[:, :, od], MAX)
                tt(tBe, Me3[:, :, 2:WP:2], Vm[:, :, 0:NP - 1], MIN)
                tt(mB[:, :, 0:W:2], Um[:, :, 0:NP - 1], tBe, MAX)
                tt(tBo, Me3[:, :, 1:WP - 2:2], Vm[:, :, 1:NP], MIN)
                tt(mB[:, :, 1:W:2], Um[:, :, 1:NP], tBo, MAX)

                # ---- final med3(A, B, C) ----
                # Lo/Me/Hi are dead once the merge pairs+completions ran;
                # alias their slots (R*WP >= R*W).
                mT = wtile("mT", R, W, tag="Lo")
                mU = wtile("mU", R, W, tag="Me")
                mV = wtile("mV", R, W, tag="Hi")
                ot = out_pool.tile([128, R * W], cdt, name="ot", tag="ot")
                ot3 = ot.rearrange("p (r w) -> p r w", w=W)
                tt(mT, mA, mB, MIN)
                tt(mU, mA, mB, MAX)
                tt(mV, mU, mC, MIN)
                tt(ot3, mT, mV, MAX)

                dma_io.dma_start(out=og[:, r0:r0 + R, :], in_=ot3[0:64])
                dma_io.dma_start(out=og[:, HH + r0:HH + r0 + R, :],
                                  in_=ot3[64:128])

    nc.compile()
    return nc


def _build_fp16_split(R=16, schedule=None, fuse=True):
    """fp16 split-plane variant: ~2x the DVE throughput of _build_shared.

    DVE TensorTensor gets the 2x_1p perf mode only when every operand is a
    2-byte dtype AND stride-1 in the last AP dim.  The even/odd horizontal
    pair sharing of _build_shared uses stride-2 APs, which forfeits the
    mode - so instead each padded row is stored as two contiguous planes
    [E | O] (E[t] = padded col 2t, O[t] = padded col 2t+1).  All 15 min/max
    ops per pixel then run stride-1 fp16 at 2 elem/cycle.

    The idle Activation engine (1.2 GHz) does the fp32->fp16 cast +
    deinterleave on load and the fp16->fp32 cast + interleave on store, so
    no DVE cycles are spent on layout or dtype conversion.

    Accuracy: fp32->fp16 rounding is monotone and median is an order
    statistic, so the result is exactly fp16(median_fp32(x)): rel err
    <= 2^-11 per element (gate is 2e-2).
    """
    import concourse.bacc as bacc
    import concourse.mybir as mybir
    from concourse.tile import TileContext

    MIN = mybir.AluOpType.min
    MAX = mybir.AluOpType.max
    f32 = mybir.dt.float32
    f16 = mybir.dt.float16
    Copy = mybir.ActivationFunctionType.Copy

    if schedule is None:
        assert HH % R == 0 and R % 2 == 0
        schedule = [R] * (HH // R)
    assert sum(schedule) == HH and all(r % 2 == 0 for r in schedule)
    Rmax = max(schedule)
    K = len(schedule)
    starts = [sum(schedule[:i]) for i in range(K)]
    NE = W // 2 + 1          # 129: plane width incl one pad col
    Hh = W // 2              # 128

    nc = bacc.Bacc("TRN2", name="median_pool2d_h")
    x = nc.dram_tensor("x", [C, H, W], f32, kind="ExternalInput")
    out = nc.dram_tensor("out", [C, H, W], f32, kind="ExternalOutput")
    xg = x.ap()
    og = out.ap()
    tt = nc.vector.tensor_tensor

    with TileContext(nc) as tc:
        with (
            tc.tile_pool(name="in32", bufs=2) as in32_pool,
            tc.tile_pool(name="in16", bufs=3) as in16_pool,
            tc.tile_pool(name="work", bufs=1) as w_pool,
            tc.tile_pool(name="o16", bufs=3) as o16_pool,
            tc.tile_pool(name="o32", bufs=2) as o32_pool,
        ):
            def wtile(name, rows, width, tag=None):
                t = w_pool.tile([128, rows * width], f16, name=name,
                                tag=tag or name)
                return t.rearrange("p (r w) -> p r w", w=width)

            def dma_phase(k):
                """Issue chunk k's input DMAs (f32 staging)."""
                r0, R = starts[k], schedule[k]
                raw = in32_pool.tile([128, (R + 2) * W], f32, name="raw",
                                     tag="raw")
                raw3 = raw.rearrange("p (r w) -> p r w", w=W)
                it = in16_pool.tile([128, (R + 2) * WP], f16, name="it",
                                    tag="it")
                it3 = it.rearrange("p (r w) -> p r w", w=WP)

                # Top/bottom halves with halo clip; at the image borders the
                # halo row is zeroed in the f32 staging tile (disjoint from
                # the DMA region, so it runs immediately) and the cast then
                # produces fp16 zeros - no post-cast fixup on the DVE.
                # Halves go out on different HWDGE queues (SP + Act) so the
                # descriptor gen and transfers run in parallel; memsets run
                # on the otherwise-idle GPSIMD, not the critical-path DVE.
                if k == 0:
                    nc.gpsimd.memset(raw3[0:64, 0:1, :], 0.0)
                    nc.sync.dma_start(out=raw3[0:64, 1:R + 2, :],
                                      in_=xg[:, 0:R + 1, :])
                else:
                    nc.sync.dma_start(out=raw3[0:64, :, :],
                                      in_=xg[:, r0 - 1:r0 + R + 1, :])
                if k == K - 1:
                    nc.gpsimd.memset(raw3[64:128, R + 1:R + 2, :], 0.0)
                    nc.scalar.dma_start(out=raw3[64:128, 0:R + 1, :],
                                        in_=xg[:, HH + r0 - 1:H, :])
                else:
                    nc.scalar.dma_start(out=raw3[64:128, :, :],
                                        in_=xg[:, HH + r0 - 1:HH + r0 + R + 1, :])
                return raw3, it3

            def cast_phase(raw3, it3, split_rows=0):
                # cast + deinterleave on Act:
                # input col j <-> padded col j+1: odd j -> E[(j+1)/2],
                # even j -> O[j/2].  E[0] and O[128] are the zero pads.
                # One fused instruction: plane outer dim reads input col
                # starts {1, 0} (stride -1) and writes plane starts {1, NE}.
                # split_rows > 0 emits the first split_rows rows as their own
                # instruction (chunk 0: the DVE's first ops need only rows
                # 1..2, so they can start under the remainder of the cast).
                if fuse:
                    nrows = raw3.shape[1]
                    parts = ([(0, split_rows), (split_rows, nrows)]
                             if 0 < split_rows < nrows else [(0, nrows)])
                    for a, b in parts:
                        src = raw3[:, a:b, 1:2].copy()
                        part = [list(p) for p in src.ap[:-1]]
                        src.ap[:] = part + [[-1, 2], [2, Hh]]
                        dst = it3[:, a:b, 1:2].copy()
                        part = [list(p) for p in dst.ap[:-1]]
                        dst.ap[:] = part + [[NE - 1, 2], [1, Hh]]
                        nc.scalar.activation(out=dst, in_=src, func=Copy)
                else:
                    nc.scalar.activation(out=it3[:, :, 1:NE],
                                         in_=raw3[:, :, 1:W:2], func=Copy)
                    nc.scalar.activation(out=it3[:, :, NE:WP - 1],
                                         in_=raw3[:, :, 0:W:2], func=Copy)
                nc.gpsimd.memset(it3[:, :, 0:WP:WP - 1], 0.0)
                return it3

            def input_phase(k):
                """DMA chunk k (f32) and cast+deinterleave it to fp16.

                Issued one chunk ahead of the DVE work so the Act engine's
                in-order stream never makes the DVE wait: cast_in(k+1) is
                issued (and runs) before cast_out(k).
                """
                return cast_phase(*dma_phase(k))

            # Ramp ordering: chunk-0's DMAs go out first (both HWDGE queues
            # start immediately), the activation-table load (1.28us, lazily
            # inserted before the first Activation = the prewarm copy below)
            # then overlaps the transfers, and the chunk-0 cast follows.
            raw3_0, it3_0 = dma_phase(0)
            warm = w_pool.tile([128, 2], f16, name="warm", tag="warm")
            nc.gpsimd.memset(warm[:], 0.0)
            nc.scalar.activation(out=warm[:, 1:2], in_=warm[:, 0:1], func=Copy)
            it3_next = cast_phase(raw3_0, it3_0)

            for k in range(K):
                r0, R = starts[k], schedule[k]
                Rh = R // 2
                it3 = it3_next
                if k + 1 < K:
                    it3_next = input_phase(k + 1)

                # ---- vertical sort3, row-pair shared (plane-agnostic) ----
                Lo = wtile("Lo", R, WP)
                Me = wtile("Me", R, WP)
                Hi = wtile("Hi", R, WP)
                if fuse:
                    # The pad columns' vertical outputs are always zero (the
                    # inputs there are the zero-pad), so the vertical runs on
                    # the 256 interior columns only (contiguous in plane
                    # layout: cols 1..256 = E[1..128]+O[0..127]) and the idle
                    # GPSIMD zeroes the Lo/Me/Hi pad cols - they must be
                    # re-zeroed every chunk because mT/mU/mV alias the slots.
                    WI = W
                    nc.gpsimd.memset(Lo[:, :, 0:WP:WP - 1], 0.0)
                    nc.gpsimd.memset(Me[:, :, 0:WP:WP - 1], 0.0)
                    nc.gpsimd.memset(Hi[:, :, 0:WP:WP - 1], 0.0)
                    Pm = wtile("Pm", Rh, WI)
                    PM = wtile("PM", Rh, WI)
                    tt(out=Pm, in0=it3[:, 1:R + 1:2, 1:WP - 1],
                       in1=it3[:, 2:R + 2:2, 1:WP - 1], op=MIN)
                    tt(out=PM, in0=it3[:, 1:R + 1:2, 1:WP - 1],
                       in1=it3[:, 2:R + 2:2, 1:WP - 1], op=MAX)

                    # Even/odd output rows fused into single instructions via
                    # custom access patterns (halves the per-instruction
                    # SBUF-access overhead).  Third element rows: even out
                    # rows use in-tile row 2i, odd rows use 2i+3 - expressed
                    # as an outer dim of stride 3 rows x 2.  Out rows
                    # {0,2,..} + {1,3,..}: outer stride 1 row x 2.  Pm/PM are
                    # repeated across the outer dim with stride 0.
                    def rows2(base3, outer_stride_rows, inner_stride_rows,
                              pitch, col0):
                        v = base3[:, 0:1, col0:col0 + 1].copy()
                        part = [list(p) for p in v.ap[:-2]]
                        v.ap[:] = part + [
                            [outer_stride_rows * pitch, 2],
                            [inner_stride_rows * pitch, Rh], [1, WI]]
                        return v

                    a_eo = rows2(it3, 3, 2, WP, 1)    # rows {0,2,..},{3,5,..}
                    PmR = rows2(Pm, 0, 1, WI, 0)      # rows {0..Rh-1} twice
                    PMR = rows2(PM, 0, 1, WI, 0)
                    Lo_eo = rows2(Lo, 1, 2, WP, 1)    # rows {0,2,..},{1,3,..}
                    Hi_eo = rows2(Hi, 1, 2, WP, 1)
                    Me_eo = rows2(Me, 1, 2, WP, 1)
                    tEO = wtile("tEO", R, WI)
                    tEOv = tEO.rearrange("p (a r) w -> p a r w", a=2)
                    tt(out=Lo_eo, in0=a_eo, in1=PmR, op=MIN)
                    tt(out=Hi_eo, in0=a_eo, in1=PMR, op=MAX)
                    tt(out=tEOv, in0=a_eo, in1=PMR, op=MIN)
                    tt(out=Me_eo, in0=PmR, in1=tEOv, op=MAX)
                else:
                    Pm = wtile("Pm", Rh, WP)
                    PM = wtile("PM", Rh, WP)
                    tt(out=Pm, in0=it3[:, 1:R + 1:2, :],
                       in1=it3[:, 2:R + 2:2, :], op=MIN)
                    tt(out=PM, in0=it3[:, 1:R + 1:2, :],
                       in1=it3[:, 2:R + 2:2, :], op=MAX)
                    tE = wtile("tE", Rh, WP)
                    tO = wtile("tO", Rh, WP)
                    a_e = it3[:, 0:R:2, :]
                    a_o = it3[:, 3:R + 2:2, :]
                    tt(out=Lo[:, 0:R:2], in0=a_e, in1=Pm, op=MIN)
                    tt(out=Hi[:, 0:R:2], in0=a_e, in1=PM, op=MAX)
                    tt(out=tE, in0=a_e, in1=PM, op=MIN)
                    tt(out=Me[:, 0:R:2], in0=Pm, in1=tE, op=MAX)
                    tt(out=Lo[:, 1:R:2], in0=a_o, in1=Pm, op=MIN)
                    tt(out=Hi[:, 1:R:2], in0=a_o, in1=PM, op=MAX)
                    tt(out=tO, in0=a_o, in1=PM, op=MIN)
                    tt(out=Me[:, 1:R:2], in0=Pm, in1=tO, op=MAX)

                # ---- horizontal merge on planes, all stride-1 ----
                LoE, LoO = Lo[:, :, 0:NE], Lo[:, :, NE:WP]
                HiE, HiO = Hi[:, :, 0:NE], Hi[:, :, NE:WP]
                MeE, MeO = Me[:, :, 0:NE], Me[:, :, NE:WP]
                # out col 2t   = window {E[t], O[t], E[t+1]}  (pair t)
                # out col 2t+1 = window {O[t], E[t+1], O[t+1]} (pair t+1)
                pA = wtile("pA", R, NE)
                mA = wtile("mA", R, W)
                pC = wtile("pC", R, NE)
                mC = wtile("mC", R, W)
                Um = wtile("Um", R, NE)
                Vm = wtile("Vm", R, NE)
                mB = wtile("mB", R, W)
                tt(out=pA, in0=LoE, in1=LoO, op=MAX)
                tt(out=pC, in0=HiE, in1=HiO, op=MIN)
                tt(out=Um, in0=MeE, in1=MeO, op=MIN)
                tt(out=Vm, in0=MeE, in1=MeO, op=MAX)
                if fuse:
                    # E and O output planes fused per instruction: the pair
                    # operand reads cols {0..Hh-1} then {1..Hh} (outer dim of
                    # stride 1 x 2); the third operand is plane-layout cols
                    # 1..W (contiguous); out planes are contiguous [E | O].
                    def cols2(base3):
                        v = base3[:, :, 0:1].copy()
                        part = [list(p) for p in v.ap[:-2]]
                        rs = list(v.ap[-2])
                        v.ap[:] = part + [rs, [1, 2], [1, Hh]]
                        return v

                    def planes(t3, lo=0):
                        return t3[:, :, lo:lo + W].rearrange(
                            "p r (a b) -> p r a b", a=2)

                    tB = wtile("tB", R, W)
                    tt(out=planes(mA), in0=cols2(pA), in1=planes(Lo, 1),
                       op=MAX)
                    tt(out=planes(mC), in0=cols2(pC), in1=planes(Hi, 1),
                       op=MIN)
                    tt(out=planes(tB), in0=cols2(Vm), in1=planes(Me, 1),
                       op=MIN)
                    tt(out=planes(mB), in0=cols2(Um), in1=planes(tB),
                       op=MAX)
                else:
                    tBe = wtile("tBe", R, Hh)
                    tBo = wtile("tBo", R, Hh)
                    tt(out=mA[:, :, 0:Hh], in0=pA[:, :, 0:Hh],
                       in1=LoE[:, :, 1:NE], op=MAX)
                    tt(out=mA[:, :, Hh:W], in0=pA[:, :, 1:NE],
                       in1=LoO[:, :, 0:Hh], op=MAX)
                    tt(out=mC[:, :, 0:Hh], in0=pC[:, :, 0:Hh],
                       in1=HiE[:, :, 1:NE], op=MIN)
                    tt(out=mC[:, :, Hh:W], in0=pC[:, :, 1:NE],
                       in1=HiO[:, :, 0:Hh], op=MIN)
                    tt(out=tBe, in0=Vm[:, :, 0:Hh], in1=MeE[:, :, 1:NE],
                       op=MIN)
                    tt(out=mB[:, :, 0:Hh], in0=Um[:, :, 0:Hh], in1=tBe,
                       op=MAX)
                    tt(out=tBo, in0=Vm[:, :, 1:NE], in1=MeO[:, :, 0:Hh],
                       op=MIN)
                    tt(out=mB[:, :, Hh:W], in0=Um[:, :, 1:NE], in1=tBo,
                       op=MAX)

                # ---- final med3(A, B, C) in plane layout ----
                # Lo/Me/Hi are dead after the merge; alias their slots.
                mT = wtile("mT", R, W, tag="Lo")
                mU = wtile("mU", R, W, tag="Me")
                mV = wtile("mV", R, W, tag="Hi")
                tt(out=mT, in0=mA, in1=mB, op=MIN)
                tt(out=mU, in0=mA, in1=mB, op=MAX)
                tt(out=mV, in0=mU, in1=mC, op=MIN)

                o32 = o32_pool.tile([128, R * W], f32, name="o32", tag="o32")
                o323 = o32.rearrange("p (r w) -> p r w", w=W)
                if fuse and k == K - 1:
                    # Tail trim: the last chunk's final op writes the fp32
                    # output directly with an interleaving AP (f32 out drops
                    # the 2x mode, but the chunk is tiny), removing the Act
                    # out-cast from the drain-critical path - both output
                    # DMAs then start right after the DVE's last op.
                    dst = o323[:, :, 0:1].copy()
                    part = [list(p) for p in dst.ap[:-2]]
                    rs = list(dst.ap[-2])
                    dst.ap[:] = part + [rs, [1, 2], [2, Hh]]
                    tt(out=dst,
                       in0=mT[:, :, 0:W].rearrange("p r (a b) -> p r a b",
                                                   a=2),
                       in1=mV[:, :, 0:W].rearrange("p r (a b) -> p r a b",
                                                   a=2),
                       op=MAX)
                else:
                    ot = o16_pool.tile([128, R * W], f16, name="ot", tag="ot")
                    ot3 = ot.rearrange("p (r w) -> p r w", w=W)
                    tt(out=ot3, in0=mT, in1=mV, op=MAX)

                    # ---- interleave + cast out on Act ----
                    if fuse:
                        dst = o323[:, :, 0:1].copy()
                        part = [list(p) for p in dst.ap[:-1]]
                        dst.ap[:] = part + [[1, 2], [2, Hh]]
                        nc.scalar.activation(
                            out=dst,
                            in_=ot3[:, :, 0:W].rearrange(
                                "p r (a b) -> p r a b", a=2),
                            func=Copy)
                    else:
                        nc.scalar.activation(out=o323[:, :, 0:W:2],
                                             in_=ot3[:, :, 0:Hh], func=Copy)
                        nc.scalar.activation(out=o323[:, :, 1:W:2],
                                             in_=ot3[:, :, Hh:W], func=Copy)
                nc.sync.dma_start(out=og[:, r0:r0 + R, :], in_=o323[0:64])
                nc.scalar.dma_start(out=og[:, HH + r0:HH + r0 + R, :],
                                    in_=o323[64:128])

    nc.compile()
    return nc


def _build_copy():
    """Calibration kernel: pure DMA passthrough x -> out."""
    import concourse.bacc as bacc
    import concourse.mybir as mybir
    from concourse.tile import TileContext

    f32 = mybir.dt.float32
    nc = bacc.Bacc("TRN2", name="median_copy_cal")
    x = nc.dram_tensor("x", [C, H, W], f32, kind="ExternalInput")
    out = nc.dram_tensor("out", [C, H, W], f32, kind="ExternalOutput")
    xf = x.ap().rearrange("c h w -> (c h) w").rearrange(
        "(n p) w -> n p w", p=128)
    of = out.ap().rearrange("c h w -> (c h) w").rearrange(
        "(n p) w -> n p w", p=128)
    n = xf.shape[0]
    with TileContext(nc) as tc:
        with tc.tile_pool(name="io", bufs=4) as pool:
            for i in range(0, n, 8):
                t = pool.tile([128, 8 * W], f32, name="t", tag="t")
                t3 = t.rearrange("p (n w) -> p n w", w=W)
                nc.sync.dma_start(out=t3[:], in_=xf[i:i + 8].rearrange(
                    "n p w -> p n w"))
                nc.sync.dma_start(out=of[i:i + 8].rearrange("n p w -> p n w"),
                                  in_=t3[:])
    nc.compile()
    return nc


def _get_nc(R=8, gp_rows=0, shared=False, gp_frac=0.0, copy=False,
            dtype="float32", in_bufs=None, out_bufs=None, fp16split=False,
            schedule=None, fuse=True):
    key = (R, gp_rows, shared, gp_frac, copy, dtype, in_bufs, out_bufs,
           fp16split, tuple(schedule) if schedule else None, fuse)
    if key not in _CACHE:
        if copy:
            _CACHE[key] = _build_copy()
        elif fp16split:
            _CACHE[key] = _build_fp16_split(R=R, schedule=schedule, fuse=fuse)
        elif shared:
            _CACHE[key] = _build_shared(R=R, gp_frac=gp_frac, dtype=dtype,
                                        in_bufs=in_bufs, out_bufs=out_bufs)
        else:
            _CACHE[key] = _build(R=R, gp_rows=gp_rows)
    return _CACHE[key]


BEST_SCHEDULE = [2, 6] + [16] * 7 + [6, 2]


def _get_best_nc():
    """The module kernel() runs: fp16 split-plane, with fallbacks."""
    try:
        return _get_nc(fp16split=True, schedule=BEST_SCHEDULE)
    except Exception:
        pass
    try:
        return _get_nc(fp16split=True, R=16, fuse=False)
    except Exception:
        pass
    try:
        return _get_nc(shared=True, R=16)
    except Exception:
        return _get_nc(R=8)


def kernel(x: np.ndarray) -> np.ndarray:
    """MedianPool2d(3x3, s=1, p=1) on 8 NeuronCores.  Result is exactly
    fp16(median_fp32(x)) - fp32->fp16 rounding is monotone and the median
    is an order statistic, so rounding commutes with the selection: per-
    element rel err <= 2^-11, l2 rel err ~2e-4 (fp32 fallbacks are bit-
    exact)."""
    from concourse.bass_utils import run_bass_kernel_spmd

    assert x.shape == (B, C, H, W), x.shape
    x = np.ascontiguousarray(x, dtype=np.float32)
    nc = _get_best_nc()
    in_maps = [{"x": x[i]} for i in range(NCORES)]
    res = run_bass_kernel_spmd(nc, in_maps, core_ids=list(range(NCORES)))
    return np.stack([r["out"] for r in res.results], axis=0)

